# revision 44
# baseline (speedup 1.0000x reference)
"""Single-head causal attention (B=4, S=4096, E=1024, D=64) on 8 TRN2 NeuronCores.

Sharding: 8 cores = 4 batches x 2 roles. Within a batch, query rows are dealt
to the two cores in interleaved 256-row blocks (role r owns global blocks
2i+r, i=0..7). Both cores project the full K/V of their batch; activations
are encoded host-side as fp16 (inputs are ~N(0,1): fp16 quantization adds
~5e-4 relative error, far below the bf16 attention pipeline's own error),
halving HBM traffic to 20 MiB/core. No collectives: cross-core sync points
expose multi-10us core start skew on this runtime, so each core runs fully
independently.

The host passes activations transposed (E-major) in 1024-row super-tiles so
projections run as 1024-wide fp16 matmuls (half the instruction count of
512-wide fp32). Attention runs in bf16 with f32 PSUM accumulation, identical
causal geometry for both roles; per-core 0/1 mask inputs enforce causality
inside diagonal tiles. Softmax skips max-subtraction (|scores/8| < ~6 for
this data) and gets the denominator from a ones-column appended to V.

Emission interleaves attention (s,t) groups into the projection stream by
estimated ready-time so the PE never drains. Input DMAs ride the sync HWDGE
queue exclusively; weights/masks/biases ride gpsimd (SWDGE) and the v
transposes ride the ACT HWDGE ring, so nothing delays an input tile. The
output is stored in a partition-contiguous [128, 16, 64] layout (one DMA
per query pair) and unsharded on the host. Per-pair output accumulators
live in SBUF (PSUM is fully subscribed: 2 projection + 4 score + 2
attnv/finalize banks); each attnv group lands in a rotating PSUM bank and
is added on DVE.
"""

import numpy as np
import ml_dtypes

import concourse.bass as bass
import concourse.tile as tile
from concourse import bacc, mybir
from concourse.bass_utils import run_bass_kernel_spmd
from concourse.masks import make_identity

B, S, E, QD = 4, 4096, 1024, 64
N_CORES = 8
QBLK = 256            # query rows per block
NBLK = 8              # blocks per core
SQ = QBLK * NBLK      # 2048 query rows per core
KV_TILE = 512
SUP = 1024            # projection super-tile rows
F32 = mybir.dt.float32
BF16 = mybir.dt.bfloat16
FP16 = mybir.dt.float16
ACTF = mybir.ActivationFunctionType


def build_nc():
    nc = bacc.Bacc(trn_type="TRN2", num_devices=N_CORES)

    # activations arrive super-tile-major: [sup, partition, e-chunk, col] so
    # each 1024-col projection super-tile is one DMA of 128 x 16KB lines
    xqT = nc.dram_tensor("xqT", [SQ // SUP, 128, 8, SUP], FP16,
                         kind="ExternalInput")
    xkT = nc.dram_tensor("xkT", [S // SUP, 128, 8, SUP], FP16,
                         kind="ExternalInput")
    xvT = nc.dram_tensor("xvT", [S // SUP, 128, 8, SUP], FP16,
                         kind="ExternalInput")
    wqT = nc.dram_tensor("wqT", [E, QD], FP16, kind="ExternalInput")
    wkT = nc.dram_tensor("wkT", [E, QD], FP16, kind="ExternalInput")
    wvT = nc.dram_tensor("wvT", [E, QD], FP16, kind="ExternalInput")
    bqkv = nc.dram_tensor("bqkv", [QD, 3], F32, kind="ExternalInput")
    masks = nc.dram_tensor("masks", [128, 8, KV_TILE], BF16, kind="ExternalInput")
    # out[p, 4s+u, d] = output row 512s+128u+p (partition-contiguous store;
    # a row-major [SQ, QD] layout would force 256B store descriptors)
    out = nc.dram_tensor("out", [128, SQ // 128, QD], F32, kind="ExternalOutput")

    with tile.TileContext(nc) as tc:
        with (
            tc.tile_pool(name="consts", bufs=1) as consts,
            tc.tile_pool(name="xin", bufs=8) as xin,
            tc.tile_pool(name="persist", bufs=1) as persist,
            tc.tile_pool(name="vtmp", bufs=2) as vtmp,
            tc.tile_pool(name="expp", bufs=10) as expp,
            tc.tile_pool(name="fin", bufs=4) as fin,
            tc.tile_pool(name="pproj", bufs=2, space="PSUM") as pproj,
            tc.tile_pool(name="psc", bufs=2, space="PSUM") as psc,
            tc.tile_pool(name="pav", bufs=2, space="PSUM") as pav,
        ):
            # ---- constants ----
            # weights come host-side pre-arranged as [128, 8, 64] (partition-
            # contiguous) so the DMA is 128 x 1KB descriptors; w_k first (it
            # gates the first projection matmul), and all constants ride
            # gpsimd (SWDGE) so the sync HWDGE FIFO carries only input tiles
            w_sb = {}
            for nm, th in (("k", wkT), ("v", wvT), ("q", wqT)):
                w = consts.tile([128, 8, QD], FP16, name=f"w_{nm}")
                nc.gpsimd.dma_start(
                    out=w, in_=th[:, :].rearrange("(p e) d -> p e d", p=128)
                )
                w_sb[nm] = w
            # biases + masks ride gpsimd (SWDGE) so they don't delay the
            # first kv input tile in the sync HWDGE FIFO; the mask DMA is
            # emitted later (first needed by the third attention batch)
            bqkv_sb = consts.tile([QD, 3], F32)
            nc.gpsimd.dma_start(out=bqkv_sb, in_=bqkv[:, :])
            b_sb = {
                "q": bqkv_sb[:, 0:1],
                "k": bqkv_sb[:, 1:2],
                "v": bqkv_sb[:, 2:3],
            }
            mask_sb = consts.tile([128, 8, KV_TILE], BF16)
            ident_f = consts.tile([128, 128], F32)
            make_identity(nc, ident_f)

            # ---- persistent projected tensors ----
            qT_sb = persist.tile([QD, SQ], BF16)          # [64, 2048]
            kT_sb = persist.tile([QD, S], BF16)           # [64, 4096]
            v_sb = persist.tile([128, S // 128, QD + 1], BF16)  # [128, 32, 65]
            nc.vector.memset(v_sb[:, :, QD : QD + 1], 1.0)

            def prefetch(xT, s2, half_order=(0, 1)):
                """Issue a super-tile's input DMAs now; project later."""
                xt = xin.tile([128, 8, SUP], FP16, name="xt", tag="xin")
                for h in half_order:
                    nc.sync.dma_start(
                        out=xt[:, :, 512 * h : 512 * h + 512],
                        in_=xT[s2, :, :, 512 * h : 512 * h + 512],
                    )
                return xt

            def project(w, xT, s2, emit_half, half_order=(0, 1), xt=None):
                """Project super-tile s2 (fp16) in two 512-col halves, each
                into its own 1-bank PSUM tile (matmul output is capped at
                one bank, and half-granular tiles let the DVE bias-add of
                one half overlap the matmuls of the next). The input DMA is
                split per half so the first half's matmuls start ~6us before
                the full super-tile has landed; half_order=(1, 0) makes the
                high half (e.g. the deepest q pair) available first."""
                if xt is None:
                    xt = prefetch(xT, s2, half_order)
                for h in half_order:
                    ps = pproj.tile([QD, KV_TILE], F32, tag="pproj")
                    for e in range(8):
                        nc.tensor.matmul(
                            ps,
                            lhsT=w[:, e, :],
                            rhs=xt[:, e, 512 * h : 512 * h + 512],
                            start=(e == 0),
                            stop=(e == 7),
                        )
                    emit_half(h, ps)

            def project_q(s2):
                """Project q super-tile s2 (pairs 2*s2, 2*s2+1); the deeper
                pair (high half) is projected first — it has more attention
                groups downstream."""
                def half(h, ps):
                    c0 = SUP * s2 + 512 * h
                    nc.vector.tensor_scalar_add(
                        out=qT_sb[:, c0 : c0 + 512], in0=ps, scalar1=b_sb["q"]
                    )
                project(w_sb["q"], xqT, s2, half, half_order=(1, 0))

            def project_k(s2):
                """Project k super-tile s2 (kv tiles 2*s2, 2*s2+1)."""
                def half(h, ps):
                    c0 = SUP * s2 + 512 * h
                    nc.vector.tensor_scalar_add(
                        out=kT_sb[:, c0 : c0 + 512], in0=ps, scalar1=b_sb["k"]
                    )
                project(w_sb["k"], xkT, s2, half)

            vt_of = {}

            def project_v(s2, xt=None):
                vt = vtmp.tile([QD, SUP], BF16, tag="vtmp")

                def half(h, ps):
                    nc.vector.tensor_scalar_add(
                        out=vt[:, 512 * h : 512 * (h + 1)], in0=ps,
                        scalar1=b_sb["v"],
                    )
                project(w_sb["v"], xvT, s2, half, xt=xt)
                vt_of[s2] = vt

            def emit_v_transpose(s2):
                """vT -> v via xbar DMA transpose (col-block j of vt lands on
                partitions as chunk j); the xbar writes 256B tiles so it must
                target a contiguous temp, not v_sb's 65-strided rows. Issued
                from the ACT sequencer (can't stall the sync input FIFO) and
                emitted mid-attention-batch so the issue never waits on vt."""
                vtT = vtmp.tile([128, 8, QD], BF16, tag="vtT")
                nc.scalar.dma_start_transpose(out=vtT, in_=vt_of[s2][:, :])
                nc.vector.tensor_copy(
                    out=v_sb[:, 8 * s2 : 8 * (s2 + 1), 0:QD], in_=vtT
                )

            # ---- attention ----
            oT_of = {}
            started = {}
            for s in range(4):
                oT_of[s] = persist.tile(
                    [QD + 1, KV_TILE], F32, name=f"oT{s}"
                )
                started[s] = False

            def emit_sc_group(s, t):
                """scores+exp for the 4 kv chunks of kv tile t in pair s;
                returns (a, ex_ap, col0) quadruples. Diagonal chunks with
                j >= 4 only concern block 2s+1 (right 256 columns), so
                scores/exp/attnv all run half-width there."""
                exs = []
                for half in range(2):
                    a0 = 4 * t + 2 * half
                    j0 = a0 - 8 * s
                    col0 = 256 if j0 >= 4 else 0
                    w = KV_TILE - col0
                    rhs_q = qT_sb[:, 512 * s + col0 : 512 * (s + 1)]
                    sc = psc.tile([128, 2, KV_TILE], F32, tag="psc")
                    for q in range(2):
                        nc.tensor.matmul(
                            sc[:, q, 0:w],
                            lhsT=kT_sb[:, 128 * (a0 + q) : 128 * (a0 + q + 1)],
                            rhs=rhs_q,
                            start=True,
                            stop=True,
                        )
                    ex = expp.tile([128, 2, KV_TILE], BF16, tag="expp")
                    nc.scalar.activation(
                        out=ex[:, :, 0:w], in_=sc[:, :, 0:w],
                        func=ACTF.Exp, scale=0.125,
                    )
                    if j0 >= 0:
                        nc.vector.tensor_mul(
                            ex[:, :, 0:w], ex[:, :, 0:w],
                            mask_sb[:, j0 : j0 + 2, col0:KV_TILE],
                        )
                    exs.append((a0, ex[:, 0, 0:w], col0))
                    exs.append((a0 + 1, ex[:, 1, 0:w], col0))
                return exs

            def emit_av_group(s, exs):
                oT = oT_of[s]
                col0 = exs[0][2]
                pv = pav.tile([QD + 1, KV_TILE], F32, tag="pav")
                for idx, (a, ex, c0) in enumerate(exs):
                    nc.tensor.matmul(
                        pv[:, c0:KV_TILE],
                        lhsT=v_sb[:, a, :],
                        rhs=ex,
                        start=idx == 0,
                        stop=idx == len(exs) - 1,
                    )
                if started[s]:
                    nc.vector.tensor_add(
                        oT[:, col0:KV_TILE], oT[:, col0:KV_TILE],
                        pv[:, col0:KV_TILE],
                    )
                else:
                    nc.vector.tensor_copy(
                        out=oT[:, col0:KV_TILE], in_=pv[:, col0:KV_TILE]
                    )
                    started[s] = True

            # per-pair output staging: normalized blocks accumulate here and
            # ship as ONE sync DMA per pair at the end of emission (16 small
            # SWDGE out-DMAs cost ~0.8us serial issue each on gpsimd)
            stg_of = {
                s: fin.tile([128, 4, QD], F32, name=f"stg{s}") for s in range(4)
            }

            def finalize_half(s, hb):
                """Normalize 256 output columns (hb=0: left block 2s, hb=1:
                right block 2s+1) once their accumulation is final."""
                oT = oT_of[s]
                for uu in range(2):
                    u = 2 * hb + uu
                    pt = pav.tile([128, QD + 1], F32, tag="pav", name="pt")
                    nc.tensor.transpose(
                        pt,
                        oT[:, 128 * u : 128 * (u + 1)],
                        ident_f[: QD + 1, : QD + 1],
                    )
                    rec = fin.tile([128, 1], F32, tag="rec")
                    nc.vector.reciprocal(rec, pt[:, QD : QD + 1])
                    nc.vector.tensor_scalar_mul(
                        stg_of[s][:, u, :], pt[:, 0:QD], rec
                    )

            def store_pair(s):
                nc.sync.dma_start(out=out[:, 4 * s : 4 * (s + 1), :], in_=stg_of[s])

            pending = []

            def emit_av_pending(ps_, pt_, exs_):
                emit_av_group(ps_, exs_)
                # left columns are final once the last full-width group
                # (tile 2s) is accumulated; right after tile 2s+1
                if pt_ == 2 * ps_:
                    finalize_half(ps_, 0)
                if pt_ == 2 * ps_ + 1:
                    finalize_half(ps_, 1)

            def att_groups(groups):
                # attnv lags its scores by TWO groups: by the time the PE
                # reaches an av group, its exp finished ~a group ago, so the
                # PE never stalls on ACT (which runs ~1.9us/group vs the
                # PE's ~1.7us of fills)
                for s, t in groups:
                    exs = emit_sc_group(s, t)
                    if len(pending) >= 2:
                        emit_av_pending(*pending.pop(0))
                    pending.append((s, t, exs))

            def att_flush():
                while pending:
                    emit_av_pending(*pending.pop(0))

            # ---- emission: interleave by estimated ready-time ----
            # kv super-tile s2 provides kv tiles {2*s2, 2*s2+1}; q super 1
            # is loaded first so the deep pairs (3: 8 groups, 2: 6 groups)
            # start early and the shallow pairs form the tail. v transposes
            # are emitted one sc-group into the following attention batch so
            # their ACT-queue issue never waits on the projection.
            #
            project_k(0)
            xt_v0 = prefetch(xvT, 0)
            project_q(1)
            att_groups([(3, 0)])
            project_v(0, xt=xt_v0)
            # with the two-group attnv lag, av(3,0) is emitted two score
            # groups later — after the transpose below
            emit_v_transpose(0)
            att_groups([(3, 1), (2, 0), (2, 1)])
            project_k(1)
            project_v(1)
            nc.gpsimd.dma_start(out=mask_sb, in_=masks[:, :, :])
            att_groups([(3, 2)])
            emit_v_transpose(1)
            att_groups([(3, 3), (2, 2), (2, 3)])
            project_k(2)
            project_v(2)
            att_groups([(3, 4)])
            emit_v_transpose(2)
            att_groups([(3, 5), (2, 4), (2, 5)])
            project_q(0)
            att_groups([(1, 0), (1, 1), (0, 0), (0, 1), (1, 2), (1, 3)])
            project_k(3)
            project_v(3)
            att_groups([(3, 6)])
            emit_v_transpose(3)
            att_groups([(3, 7)])
            att_flush()
            store_pair(2)
            store_pair(0)
            store_pair(1)
            store_pair(3)

    nc.compile()
    return nc


def shard_inputs(query, key, value, Wq, bq, Wk, bk, Wv, bv):
    """Build per-core input maps (host-side sharding: slice/transpose/fp16)."""
    query = np.asarray(query, dtype=np.float32)
    key = np.asarray(key, dtype=np.float32)
    value = np.asarray(value, dtype=np.float32)

    def w_arrange(W):
        # device reads weight row (8p + e) as (partition p, e-chunk e);
        # original E index is 128e + p
        wT = np.asarray(W, np.float32).T  # [E, QD]
        return np.ascontiguousarray(
            wT.reshape(8, 128, QD).transpose(1, 0, 2).reshape(E, QD)
        ).astype(np.float16)

    wqT = w_arrange(Wq)
    wkT = w_arrange(Wk)
    wvT = w_arrange(Wv)
    bqkv = np.ascontiguousarray(
        np.stack(
            [np.asarray(x, np.float32).reshape(QD) for x in (bq, bk, bv)],
            axis=1,
        )
    )

    # role-specific diagonal masks [128, 8, 512]:
    # col f covers block-pair: q_off = 512*(f//256) + 256*r + f%256
    # pattern j valid iff 128*j + p <= q_off
    p = np.arange(128)[:, None]
    f = np.arange(KV_TILE)[None, :]
    mask_r = []
    for r in range(2):
        q_off = 512 * (f // 256) + 256 * r + (f % 256)
        ms = np.stack(
            [(128 * j + p <= q_off) for j in range(8)], axis=1
        ).astype(ml_dtypes.bfloat16)
        mask_r.append(np.ascontiguousarray(ms))

    def tile_major(xc):
        # [C, E] -> [C/1024, 128, 8, 1024]: arr[s,p,e,c] = xc[1024s+c, 128e+p]
        C = xc.shape[0]
        return np.ascontiguousarray(
            xc.reshape(C // SUP, SUP, 8, 128).transpose(0, 3, 2, 1)
            .astype(np.float16)
        )

    in_maps = []
    for c in range(N_CORES):
        b, r = c // 2, c % 2
        rows = np.concatenate(
            [np.arange(QBLK * (2 * i + r), QBLK * (2 * i + r) + QBLK)
             for i in range(NBLK)]
        )
        xqT = tile_major(query[b][rows])                    # [2, 128, 8, 1024]
        xkT = tile_major(key[b])                            # [4, 128, 8, 1024]
        xvT = tile_major(value[b])
        in_maps.append({
            "xqT": xqT, "xkT": xkT, "xvT": xvT,
            "wqT": wqT, "wkT": wkT, "wvT": wvT,
            "bqkv": bqkv,
            "masks": mask_r[r],
        })
    return in_maps


_NC_CACHE = {}


def kernel(query, key, value, Wq, bq, Wk, bk, Wv, bv):
    if "nc" not in _NC_CACHE:
        _NC_CACHE["nc"] = build_nc()
    nc = _NC_CACHE["nc"]
    in_maps = shard_inputs(query, key, value, Wq, bq, Wk, bk, Wv, bv)
    res = run_bass_kernel_spmd(nc, in_maps, core_ids=list(range(N_CORES)))
    out = np.empty((B, S, QD), np.float32)
    for c in range(N_CORES):
        b, r = c // 2, c % 2
        # device layout [128, 16, 64]: out[p, j, :] = local row 128j + p
        o = res.results[c]["out"].transpose(1, 0, 2).reshape(SQ, QD)
        for i in range(NBLK):
            g0 = QBLK * (2 * i + r)
            out[b, g0 : g0 + QBLK] = o[QBLK * i : QBLK * (i + 1)]
    return out


# revision 45
# speedup vs baseline: 1.0632x; 1.0632x over previous
"""Single-head causal attention (B=4, S=4096, E=1024, D=64) on 8 TRN2 NeuronCores.

Sharding: 8 cores = 4 batches x 2 roles. Within a batch, query rows are dealt
to the two cores in interleaved 256-row blocks (role r owns global blocks
2i+r, i=0..7). Both cores project the full K/V of their batch; activations
are encoded host-side as fp16 (inputs are ~N(0,1): fp16 quantization adds
~5e-4 relative error, far below the bf16 attention pipeline's own error),
halving HBM traffic to 20 MiB/core. No collectives: cross-core sync points
expose multi-10us core start skew on this runtime, so each core runs fully
independently.

The host passes activations transposed (E-major) in 1024-row super-tiles so
projections run as 1024-wide fp16 matmuls (half the instruction count of
512-wide fp32). Attention runs in bf16 with f32 PSUM accumulation, identical
causal geometry for both roles; per-core 0/1 mask inputs enforce causality
inside diagonal tiles. Softmax skips max-subtraction (|scores/8| < ~6 for
this data) and gets the denominator from a ones-column appended to V.

Emission interleaves attention (s,t) groups into the projection stream by
estimated ready-time so the PE never drains. Input DMAs ride the sync HWDGE
queue exclusively; weights/masks/biases ride gpsimd (SWDGE) and the v
transposes ride the ACT HWDGE ring, so nothing delays an input tile. The
output is stored in a partition-contiguous [128, 16, 64] layout (one DMA
per query pair) and unsharded on the host. Per-pair output accumulators
live in SBUF (PSUM is fully subscribed: 2 projection + 4 score + 2
attnv/finalize banks); each attnv group lands in a rotating PSUM bank and
is added on DVE.
"""

import numpy as np
import ml_dtypes

import concourse.bass as bass
import concourse.tile as tile
from concourse import bacc, mybir
from concourse.bass_utils import run_bass_kernel_spmd
from concourse.masks import make_identity

B, S, E, QD = 4, 4096, 1024, 64
N_CORES = 8
QBLK = 256            # query rows per block
NBLK = 8              # blocks per core
SQ = QBLK * NBLK      # 2048 query rows per core
KV_TILE = 512
SUP = 1024            # projection super-tile rows
F32 = mybir.dt.float32
BF16 = mybir.dt.bfloat16
FP16 = mybir.dt.float16
ACTF = mybir.ActivationFunctionType


def build_nc():
    nc = bacc.Bacc(trn_type="TRN2", num_devices=N_CORES)

    # activations arrive super-tile-major: [sup, partition, e-chunk, col] so
    # each 1024-col projection super-tile is one DMA of 128 x 16KB lines
    xqT = nc.dram_tensor("xqT", [SQ // SUP, 128, 8, SUP], FP16,
                         kind="ExternalInput")
    xkT = nc.dram_tensor("xkT", [S // SUP, 128, 8, SUP], FP16,
                         kind="ExternalInput")
    xvT = nc.dram_tensor("xvT", [S // SUP, 128, 8, SUP], FP16,
                         kind="ExternalInput")
    wqT = nc.dram_tensor("wqT", [E, QD], FP16, kind="ExternalInput")
    wkT = nc.dram_tensor("wkT", [E, QD], FP16, kind="ExternalInput")
    wvT = nc.dram_tensor("wvT", [E, QD], FP16, kind="ExternalInput")
    bqkv = nc.dram_tensor("bqkv", [QD, 3], F32, kind="ExternalInput")
    masks = nc.dram_tensor("masks", [128, 8, KV_TILE], BF16, kind="ExternalInput")
    # out[p, 4s+u, d] = output row 512s+128u+p (partition-contiguous store;
    # a row-major [SQ, QD] layout would force 256B store descriptors)
    out = nc.dram_tensor("out", [128, SQ // 128, QD], F32, kind="ExternalOutput")

    with tile.TileContext(nc) as tc:
        with (
            tc.tile_pool(name="consts", bufs=1) as consts,
            tc.tile_pool(name="xin", bufs=8) as xin,
            tc.tile_pool(name="persist", bufs=1) as persist,
            tc.tile_pool(name="vtmp", bufs=2) as vtmp,
            tc.tile_pool(name="expp", bufs=10) as expp,
            tc.tile_pool(name="fin", bufs=4) as fin,
            tc.tile_pool(name="pproj", bufs=2, space="PSUM") as pproj,
            tc.tile_pool(name="psc", bufs=2, space="PSUM") as psc,
            tc.tile_pool(name="pav", bufs=2, space="PSUM") as pav,
        ):
            # ---- constants ----
            # weights come host-side pre-arranged as [128, 8, 64] (partition-
            # contiguous) so the DMA is 128 x 1KB descriptors; w_k first (it
            # gates the first projection matmul), and all constants ride
            # gpsimd (SWDGE) so the sync HWDGE FIFO carries only input tiles
            w_sb = {}
            for nm, th in (("k", wkT), ("v", wvT), ("q", wqT)):
                w = consts.tile([128, 8, QD], FP16, name=f"w_{nm}")
                nc.gpsimd.dma_start(
                    out=w, in_=th[:, :].rearrange("(p e) d -> p e d", p=128)
                )
                w_sb[nm] = w
            # biases + masks ride gpsimd (SWDGE) so they don't delay the
            # first kv input tile in the sync HWDGE FIFO; the mask DMA is
            # emitted later (first needed by the third attention batch)
            bqkv_sb = consts.tile([QD, 3], F32)
            nc.gpsimd.dma_start(out=bqkv_sb, in_=bqkv[:, :])
            b_sb = {
                "q": bqkv_sb[:, 0:1],
                "k": bqkv_sb[:, 1:2],
                "v": bqkv_sb[:, 2:3],
            }
            mask_sb = consts.tile([128, 8, KV_TILE], BF16)
            ident_f = consts.tile([128, 128], F32)
            make_identity(nc, ident_f)

            # ---- persistent projected tensors ----
            qT_sb = persist.tile([QD, SQ], BF16)          # [64, 2048]
            kT_sb = persist.tile([QD, S], BF16)           # [64, 4096]
            v_sb = persist.tile([128, S // 128, QD + 1], BF16)  # [128, 32, 65]
            nc.vector.memset(v_sb[:, :, QD : QD + 1], 1.0)

            def project(w, xT, s2, emit_half, half_order=(0, 1)):
                """Project super-tile s2 (fp16) in two 512-col halves, each
                into its own 1-bank PSUM tile (matmul output is capped at
                one bank, and half-granular tiles let the DVE bias-add of
                one half overlap the matmuls of the next). The input DMA is
                split per half so the first half's matmuls start ~6us before
                the full super-tile has landed; half_order=(1, 0) makes the
                high half (e.g. the deepest q pair) available first."""
                xt = xin.tile([128, 8, SUP], FP16, name="xt", tag="xin")
                for h in half_order:
                    nc.sync.dma_start(
                        out=xt[:, :, 512 * h : 512 * h + 512],
                        in_=xT[s2, :, :, 512 * h : 512 * h + 512],
                    )
                for h in half_order:
                    ps = pproj.tile([QD, KV_TILE], F32, tag="pproj")
                    for e in range(8):
                        nc.tensor.matmul(
                            ps,
                            lhsT=w[:, e, :],
                            rhs=xt[:, e, 512 * h : 512 * h + 512],
                            start=(e == 0),
                            stop=(e == 7),
                        )
                    emit_half(h, ps)

            def project_q(s2):
                """Project q super-tile s2 (pairs 2*s2, 2*s2+1); the deeper
                pair (high half) is projected first — it has more attention
                groups downstream."""
                def half(h, ps):
                    c0 = SUP * s2 + 512 * h
                    nc.vector.tensor_scalar_add(
                        out=qT_sb[:, c0 : c0 + 512], in0=ps, scalar1=b_sb["q"]
                    )
                project(w_sb["q"], xqT, s2, half, half_order=(1, 0))

            def project_k(s2):
                """Project k super-tile s2 (kv tiles 2*s2, 2*s2+1)."""
                def half(h, ps):
                    c0 = SUP * s2 + 512 * h
                    nc.vector.tensor_scalar_add(
                        out=kT_sb[:, c0 : c0 + 512], in0=ps, scalar1=b_sb["k"]
                    )
                project(w_sb["k"], xkT, s2, half)

            vt_of = {}

            def project_v(s2):
                vt = vtmp.tile([QD, SUP], BF16, tag="vtmp")

                def half(h, ps):
                    nc.vector.tensor_scalar_add(
                        out=vt[:, 512 * h : 512 * (h + 1)], in0=ps,
                        scalar1=b_sb["v"],
                    )
                project(w_sb["v"], xvT, s2, half)
                vt_of[s2] = vt

            def emit_v_transpose(s2):
                """vT -> v via xbar DMA transpose (col-block j of vt lands on
                partitions as chunk j); the xbar writes 256B tiles so it must
                target a contiguous temp, not v_sb's 65-strided rows. Issued
                from the ACT sequencer (can't stall the sync input FIFO) and
                emitted mid-attention-batch so the issue never waits on vt."""
                vtT = vtmp.tile([128, 8, QD], BF16, tag="vtT")
                nc.scalar.dma_start_transpose(out=vtT, in_=vt_of[s2][:, :])
                nc.vector.tensor_copy(
                    out=v_sb[:, 8 * s2 : 8 * (s2 + 1), 0:QD], in_=vtT
                )

            # ---- attention ----
            oT_of = {}
            started = {}
            for s in range(4):
                oT_of[s] = persist.tile(
                    [QD + 1, KV_TILE], F32, name=f"oT{s}"
                )
                started[s] = False

            def emit_sc_group(s, t):
                """scores+exp for the 4 kv chunks of kv tile t in pair s;
                returns (a, ex_ap, col0) quadruples. Diagonal chunks with
                j >= 4 only concern block 2s+1 (right 256 columns), so
                scores/exp/attnv all run half-width there."""
                exs = []
                for half in range(2):
                    a0 = 4 * t + 2 * half
                    j0 = a0 - 8 * s
                    col0 = 256 if j0 >= 4 else 0
                    w = KV_TILE - col0
                    rhs_q = qT_sb[:, 512 * s + col0 : 512 * (s + 1)]
                    sc = psc.tile([128, 2, KV_TILE], F32, tag="psc")
                    for q in range(2):
                        nc.tensor.matmul(
                            sc[:, q, 0:w],
                            lhsT=kT_sb[:, 128 * (a0 + q) : 128 * (a0 + q + 1)],
                            rhs=rhs_q,
                            start=True,
                            stop=True,
                        )
                    ex = expp.tile([128, 2, KV_TILE], BF16, tag="expp")
                    nc.scalar.activation(
                        out=ex[:, :, 0:w], in_=sc[:, :, 0:w],
                        func=ACTF.Exp, scale=0.125,
                    )
                    if j0 >= 0:
                        nc.vector.tensor_mul(
                            ex[:, :, 0:w], ex[:, :, 0:w],
                            mask_sb[:, j0 : j0 + 2, col0:KV_TILE],
                        )
                    exs.append((a0, ex[:, 0, 0:w], col0))
                    exs.append((a0 + 1, ex[:, 1, 0:w], col0))
                return exs

            def emit_av_group(s, exs):
                oT = oT_of[s]
                col0 = exs[0][2]
                pv = pav.tile([QD + 1, KV_TILE], F32, tag="pav")
                for idx, (a, ex, c0) in enumerate(exs):
                    nc.tensor.matmul(
                        pv[:, c0:KV_TILE],
                        lhsT=v_sb[:, a, :],
                        rhs=ex,
                        start=idx == 0,
                        stop=idx == len(exs) - 1,
                    )
                if started[s]:
                    nc.vector.tensor_add(
                        oT[:, col0:KV_TILE], oT[:, col0:KV_TILE],
                        pv[:, col0:KV_TILE],
                    )
                else:
                    nc.vector.tensor_copy(
                        out=oT[:, col0:KV_TILE], in_=pv[:, col0:KV_TILE]
                    )
                    started[s] = True

            # per-pair output staging: normalized blocks accumulate here and
            # ship as ONE sync DMA per pair at the end of emission (16 small
            # SWDGE out-DMAs cost ~0.8us serial issue each on gpsimd)
            stg_of = {
                s: fin.tile([128, 4, QD], F32, name=f"stg{s}") for s in range(4)
            }

            def finalize_half(s, hb):
                """Normalize 256 output columns (hb=0: left block 2s, hb=1:
                right block 2s+1) once their accumulation is final."""
                oT = oT_of[s]
                for uu in range(2):
                    u = 2 * hb + uu
                    pt = pav.tile([128, QD + 1], F32, tag="pav", name="pt")
                    nc.tensor.transpose(
                        pt,
                        oT[:, 128 * u : 128 * (u + 1)],
                        ident_f[: QD + 1, : QD + 1],
                    )
                    rec = fin.tile([128, 1], F32, tag="rec")
                    nc.vector.reciprocal(rec, pt[:, QD : QD + 1])
                    nc.vector.tensor_scalar_mul(
                        stg_of[s][:, u, :], pt[:, 0:QD], rec
                    )

            def store_pair(s):
                nc.sync.dma_start(out=out[:, 4 * s : 4 * (s + 1), :], in_=stg_of[s])

            pending = []

            def emit_av_pending(ps_, pt_, exs_):
                emit_av_group(ps_, exs_)
                # left columns are final once the last full-width group
                # (tile 2s) is accumulated; right after tile 2s+1
                if pt_ == 2 * ps_:
                    finalize_half(ps_, 0)
                if pt_ == 2 * ps_ + 1:
                    finalize_half(ps_, 1)

            def att_groups(groups):
                # attnv lags its scores by TWO groups: by the time the PE
                # reaches an av group, its exp finished ~a group ago, so the
                # PE never stalls on ACT (which runs ~1.9us/group vs the
                # PE's ~1.7us of fills)
                for s, t in groups:
                    exs = emit_sc_group(s, t)
                    if len(pending) >= 2:
                        emit_av_pending(*pending.pop(0))
                    pending.append((s, t, exs))

            def att_flush():
                while pending:
                    emit_av_pending(*pending.pop(0))

            # ---- emission: interleave by estimated ready-time ----
            # kv super-tile s2 provides kv tiles {2*s2, 2*s2+1}; q super 1
            # is loaded first so the deep pairs (3: 8 groups, 2: 6 groups)
            # start early and the shallow pairs form the tail. v transposes
            # are emitted one sc-group into the following attention batch so
            # their ACT-queue issue never waits on the projection.
            #
            project_k(0)
            project_v(0)
            project_q(1)
            # super-0's transpose goes before the first attention batch: the
            # ACT queue is idle until the first exp anyway
            emit_v_transpose(0)
            att_groups([(3, 0), (3, 1), (2, 0), (2, 1)])
            project_k(1)
            project_v(1)
            nc.gpsimd.dma_start(out=mask_sb, in_=masks[:, :, :])
            att_groups([(3, 2)])
            emit_v_transpose(1)
            att_groups([(3, 3), (2, 2), (2, 3)])
            project_k(2)
            project_v(2)
            att_groups([(3, 4)])
            emit_v_transpose(2)
            att_groups([(3, 5), (2, 4), (2, 5)])
            project_q(0)
            att_groups([(1, 0), (1, 1), (0, 0), (0, 1), (1, 2), (1, 3)])
            project_k(3)
            project_v(3)
            att_groups([(3, 6)])
            emit_v_transpose(3)
            att_groups([(3, 7)])
            att_flush()
            store_pair(2)
            store_pair(0)
            store_pair(1)
            store_pair(3)

    nc.compile()
    return nc


def shard_inputs(query, key, value, Wq, bq, Wk, bk, Wv, bv):
    """Build per-core input maps (host-side sharding: slice/transpose/fp16)."""
    query = np.asarray(query, dtype=np.float32)
    key = np.asarray(key, dtype=np.float32)
    value = np.asarray(value, dtype=np.float32)

    def w_arrange(W):
        # device reads weight row (8p + e) as (partition p, e-chunk e);
        # original E index is 128e + p
        wT = np.asarray(W, np.float32).T  # [E, QD]
        return np.ascontiguousarray(
            wT.reshape(8, 128, QD).transpose(1, 0, 2).reshape(E, QD)
        ).astype(np.float16)

    wqT = w_arrange(Wq)
    wkT = w_arrange(Wk)
    wvT = w_arrange(Wv)
    bqkv = np.ascontiguousarray(
        np.stack(
            [np.asarray(x, np.float32).reshape(QD) for x in (bq, bk, bv)],
            axis=1,
        )
    )

    # role-specific diagonal masks [128, 8, 512]:
    # col f covers block-pair: q_off = 512*(f//256) + 256*r + f%256
    # pattern j valid iff 128*j + p <= q_off
    p = np.arange(128)[:, None]
    f = np.arange(KV_TILE)[None, :]
    mask_r = []
    for r in range(2):
        q_off = 512 * (f // 256) + 256 * r + (f % 256)
        ms = np.stack(
            [(128 * j + p <= q_off) for j in range(8)], axis=1
        ).astype(ml_dtypes.bfloat16)
        mask_r.append(np.ascontiguousarray(ms))

    def tile_major(xc):
        # [C, E] -> [C/1024, 128, 8, 1024]: arr[s,p,e,c] = xc[1024s+c, 128e+p]
        C = xc.shape[0]
        return np.ascontiguousarray(
            xc.reshape(C // SUP, SUP, 8, 128).transpose(0, 3, 2, 1)
            .astype(np.float16)
        )

    in_maps = []
    for c in range(N_CORES):
        b, r = c // 2, c % 2
        rows = np.concatenate(
            [np.arange(QBLK * (2 * i + r), QBLK * (2 * i + r) + QBLK)
             for i in range(NBLK)]
        )
        xqT = tile_major(query[b][rows])                    # [2, 128, 8, 1024]
        xkT = tile_major(key[b])                            # [4, 128, 8, 1024]
        xvT = tile_major(value[b])
        in_maps.append({
            "xqT": xqT, "xkT": xkT, "xvT": xvT,
            "wqT": wqT, "wkT": wkT, "wvT": wvT,
            "bqkv": bqkv,
            "masks": mask_r[r],
        })
    return in_maps


_NC_CACHE = {}


def kernel(query, key, value, Wq, bq, Wk, bk, Wv, bv):
    if "nc" not in _NC_CACHE:
        _NC_CACHE["nc"] = build_nc()
    nc = _NC_CACHE["nc"]
    in_maps = shard_inputs(query, key, value, Wq, bq, Wk, bk, Wv, bv)
    res = run_bass_kernel_spmd(nc, in_maps, core_ids=list(range(N_CORES)))
    out = np.empty((B, S, QD), np.float32)
    for c in range(N_CORES):
        b, r = c // 2, c % 2
        # device layout [128, 16, 64]: out[p, j, :] = local row 128j + p
        o = res.results[c]["out"].transpose(1, 0, 2).reshape(SQ, QD)
        for i in range(NBLK):
            g0 = QBLK * (2 * i + r)
            out[b, g0 : g0 + QBLK] = o[QBLK * i : QBLK * (i + 1)]
    return out
